# revision 3
# baseline (speedup 1.0000x reference)
"""Fused multi-head attention block (B=16, N=1024, C=768, H=12, D=64) for 8
TRN2 NeuronCores. Data-parallel over batch: 2 batches per core, no
collectives. Per-core kernel: qkv matmul -> per-head LayerNorm -> partial
RoPE -> attention (softmax without max-subtraction; denominator fused as a
ones-column in the PV matmul) -> output projection + bias.

Matmul operands are bf16 (PE full rate); accumulation, LayerNorm statistics,
softmax denominators and the final bias-add stay fp32. Elementwise work is
batched over pairs of 128-token chunks: both DVE and ACT are
sequencer-dispatch-bound, so instruction count is the scarce resource.
Unrotated tokens ride the RoPE ops with cos=1/sin=0 padding.
"""

import os
import sys

sys.path.insert(0, "/opt/trn_rl_repo")

import numpy as np

import concourse.bass as bass
import concourse.mybir as mybir
import concourse.tile as tile
from concourse import bacc
from concourse.masks import make_identity
from concourse.bass_utils import run_bass_kernel_spmd

F32 = mybir.dt.float32
BF16 = mybir.dt.bfloat16

B_LOC = 2          # batches per core
S = 1024           # sequence length
C = 768            # model dim
H = 12             # heads
D = 64             # head dim
G = 6              # head pairs (2 heads each)
TCH = 8            # 128-token chunks per batch
NP = TCH // 2      # chunk pairs
P_TOK = 1          # num_prefix_tokens
L_TOK = 32         # num_latent_tokens
ROT = S - P_TOK - L_TOK  # 991 rotated tokens
SCALE = D ** -0.5

LAST_RESULT = None


def _bc(ap, dims):
    """Raw broadcast AP: same tensor/offset, explicit [step, count] dims."""
    return bass.AP(tensor=ap.tensor, offset=ap.offset, ap=dims)


def build_nc(reps=None):
    nc = bacc.Bacc("TRN2", target_bir_lowering=False, debug=False, num_devices=8)

    x_d = nc.declare_dram_parameter("x", [B_LOC * S, C], F32, isOutput=False)
    cos_d = nc.declare_dram_parameter("cos", [ROT, D // 2], F32, isOutput=False)
    sin_d = nc.declare_dram_parameter("sin", [ROT, D // 2], F32, isOutput=False)
    wqkv_d = nc.declare_dram_parameter("w_qkv", [C, 3 * C], F32, isOutput=False)
    wproj_d = nc.declare_dram_parameter("w_proj", [C, C], F32, isOutput=False)
    bproj_d = nc.declare_dram_parameter("b_proj", [C], F32, isOutput=False)
    out_d = nc.declare_dram_parameter("out", [B_LOC * S, C], F32, isOutput=True)

    if reps is None:
        reps = int(os.environ.get("BODY_REPS", "1"))
    with tile.TileContext(nc) as tc:
        _build_body(nc, tc, x_d, cos_d, sin_d, wqkv_d, wproj_d, bproj_d, out_d,
                    reps=reps)

    # All ACT functions used here (Exp, Ln, Square, Copy) live together in
    # the natural_log_exp_and_others table set, but the table-load pass
    # assigns each activation the first set containing its function, which
    # alternates exp/ln sets and inserts ~190 table loads (~2.7us each).
    # Present filtered tables (same order/indices) so the shared set is the
    # unique covering choice and the fixpoint pass hoists a single load.
    import concourse.bacc as bacc_mod
    used = {mybir.ActivationFunctionType.Exp, mybir.ActivationFunctionType.Ln,
            mybir.ActivationFunctionType.Square, mybir.ActivationFunctionType.Copy,
            mybir.ActivationFunctionType.Identity}
    orig_gat = bacc_mod.get_activation_tables

    def _gat(arch):
        tabs = orig_gat(arch)
        out = {}
        for name, fns in tabs.items():
            if name == "natural_log_exp_and_others":
                out[name] = fns
            else:
                out[name] = fns - used
        return out

    bacc_mod.get_activation_tables = _gat
    try:
        nc.compile()
    finally:
        bacc_mod.get_activation_tables = orig_gat
    return nc


def _build_body(nc, tc, x_d, cos_d, sin_d, wqkv_d, wproj_d, bproj_d, out_d,
                reps=1):
    from contextlib import ExitStack

    ctx = ExitStack()
    with ctx:
        singles = ctx.enter_context(tc.tile_pool(name="singles", bufs=1))
        xin_pool = ctx.enter_context(tc.tile_pool(name="xin", bufs=2))
        xt_pool = ctx.enter_context(tc.tile_pool(name="xt", bufs=2))
        at_pool = ctx.enter_context(tc.tile_pool(name="at", bufs=2))
        qs_pool = ctx.enter_context(tc.tile_pool(name="qs", bufs=2))
        qt_pool = ctx.enter_context(tc.tile_pool(name="qt", bufs=2))
        kt_pool = ctx.enter_context(tc.tile_pool(name="kt", bufs=2))
        v_pool = ctx.enter_context(tc.tile_pool(name="v", bufs=2))
        ln_pool = ctx.enter_context(tc.tile_pool(name="ln", bufs=4))
        st_pool = ctx.enter_context(tc.tile_pool(name="st", bufs=2))
        p_pool = ctx.enter_context(tc.tile_pool(name="p", bufs=3))
        ob_pool = ctx.enter_context(tc.tile_pool(name="ob", bufs=3))

        qkv_ps = ctx.enter_context(tc.tile_pool(name="qkvps", bufs=1, space="PSUM"))
        t_ps = ctx.enter_context(tc.tile_pool(name="tps", bufs=1, space="PSUM"))
        sc_ps = ctx.enter_context(tc.tile_pool(name="scps", bufs=2, space="PSUM"))
        o_ps = ctx.enter_context(tc.tile_pool(name="ops", bufs=1, space="PSUM"))

        # ---- one-time setup ----
        ident = singles.tile([128, 128], BF16)
        make_identity(nc, ident)

        eps_t = singles.tile([128, 1], F32)
        nc.vector.memset(eps_t, 1e-5)
        ln8_t = singles.tile([128, 1], F32)
        nc.vector.memset(ln8_t, -2.0794415416798357)  # ln(1/8)

        # bias as a rank-1 matmul: row-0 selector (ones in row 0) x bias row
        bias_row = singles.tile([128, C], BF16)
        nc.gpsimd.memset(bias_row, 0.0)
        nc.gpsimd.dma_start(out=bias_row[0:1, :],
                            in_=bproj_d[:].rearrange("(a f) -> a f", a=1))
        mask0 = singles.tile([128, 128], BF16)
        nc.gpsimd.memset(mask0, 0.0)
        nc.gpsimd.affine_select(
            out=mask0, in_=mask0,
            compare_op=mybir.AluOpType.not_equal,
            fill=1.0, base=0,
            pattern=[[0, 128]],
            channel_multiplier=1)

        # weights: gpsimd DMAs cast fp32 -> bf16 directly (w_qkv first — it
        # gates the first matmuls; w_proj later, it is needed only at proj)
        wq_all = singles.tile([128, G, 3 * C], BF16)
        wp_all = singles.tile([128, G, C], BF16)
        for cc in range(G):
            nc.gpsimd.dma_start(out=wq_all[:, cc, :],
                                in_=wqkv_d[cc * 128:(cc + 1) * 128, :])

        # cos/sin per chunk-PAIR: [128, 2, 32] bf16. Position p of chunk ch
        # holds angle row ch*128 + p - 1. Unrotated positions (token 0 and
        # tokens >= 992) get cos=1 / sin=0 so RoPE acts as identity there.
        # replicated over the 4 q/k head-groups so RoPE operand APs stay <=
        # 3 free dims (TENSOR3D ISA limit)
        cs_t, sn_t = [], []

        def _rep(dram, rows):
            a = dram[rows[0]:rows[1], :]
            return _bc(a, [list(a.ap[0]), [0, 4], [1, 32]])

        for cp in range(NP):
            ct = singles.tile([128, 2, 4, 32], BF16, tag=f"cos{cp}")
            st = singles.tile([128, 2, 4, 32], BF16, tag=f"sin{cp}")
            for i in range(2):
                ch = 2 * cp + i
                if ch == 0:
                    nc.vector.memset(ct[0:1, i, :, :], 1.0)
                    nc.vector.memset(st[0:1, i, :, :], 0.0)
                    nc.gpsimd.dma_start(out=ct[1:128, i, :, :], in_=_rep(cos_d, (0, 127)))
                    nc.gpsimd.dma_start(out=st[1:128, i, :, :], in_=_rep(sin_d, (0, 127)))
                elif ch == 7:
                    nc.vector.memset(ct[96:128, i, :, :], 1.0)
                    nc.vector.memset(st[96:128, i, :, :], 0.0)
                    nc.gpsimd.dma_start(out=ct[0:96, i, :, :], in_=_rep(cos_d, (895, 991)))
                    nc.gpsimd.dma_start(out=st[0:96, i, :, :], in_=_rep(sin_d, (895, 991)))
                else:
                    nc.gpsimd.dma_start(
                        out=ct[:, i, :, :], in_=_rep(cos_d, (ch * 128 - 1, ch * 128 + 127)))
                    nc.gpsimd.dma_start(
                        out=st[:, i, :, :], in_=_rep(sin_d, (ch * 128 - 1, ch * 128 + 127)))
            cs_t.append(ct)
            sn_t.append(st)

        # softmax-denominator staging + one shared broadcast-selector mask
        # (mask[k, h*64+j] = 1 iff k == h; denominator reciprocals for the
        # current head pair always sit in dnb rows 0..1)
        dn2 = singles.tile([2, S], F32)
        dnb = singles.tile([128, S], BF16)
        nc.gpsimd.memset(dnb, 0.0)
        mask1 = singles.tile([128, 128], BF16)
        nc.gpsimd.memset(mask1, 0.0)
        mk3 = mask1[:].rearrange("p (h j) -> p h j", h=2)
        nc.gpsimd.affine_select(
            out=mk3, in_=mk3,
            compare_op=mybir.AluOpType.not_equal,
            fill=1.0, base=0,
            pattern=[[-1, 2], [0, 64]],
            channel_multiplier=1)

        for cc in range(G):
            nc.gpsimd.dma_start(out=wp_all[:, cc, :],
                                in_=wproj_d[cc * 128:(cc + 1) * 128, :])

        for b in [bb for _ in range(reps) for bb in range(B_LOC)]:
            # ---- x^T (bf16) for this batch: [128(c), cc, t] ----
            xt_b = xt_pool.tile([128, G, S], BF16, tag="xt")
            for ch in range(TCH):
                xin = xin_pool.tile([128, C], F32, tag="xin")
                nc.sync.dma_start(
                    out=xin, in_=x_d[b * S + ch * 128: b * S + (ch + 1) * 128, :])
                xc = xin_pool.tile([128, C], BF16, tag="xc")
                nc.scalar.copy(xc, xin)
                for cc in range(G):
                    tp = t_ps.tile([128, 128], BF16, tag="tps")
                    nc.tensor.transpose(tp, xc[:, cc * 128:(cc + 1) * 128], ident)
                    nc.vector.tensor_copy(xt_b[:, cc, ch * 128:(ch + 1) * 128], tp)

            at_b = at_pool.tile([128, G, S], BF16, tag="at")

            for g in range(G):
                qt = qt_pool.tile([128, S], BF16, tag="qt")
                kt = kt_pool.tile([128, S], BF16, tag="kt")
                vg = v_pool.tile([128, TCH, 2, 65], BF16, tag="vg")
                nc.gpsimd.memset(vg[:, :, :, 64:65], 1.0)

                ssum_g = st_pool.tile([128, TCH, 4], F32, tag="ssum")
                ssq_g = st_pool.tile([128, TCH, 4], F32, tag="ssq")
                qkvs_l = []
                for cp in range(NP):
                    qkvs = qs_pool.tile([128, 2, 384], F32, tag=f"qkvs{cp}")
                    qkvs_l.append(qkvs)
                    for i in range(2):
                        ch = 2 * cp + i
                        qps = qkv_ps.tile([128, 384], F32, tag="qkv")
                        for cc in range(G):
                            rhs = wq_all[:, cc, :].rearrange(
                                "p (t g j) -> p t g j", t=3, j=128)[:, :, g, :]
                            nc.tensor.matmul(
                                qps,
                                lhsT=xt_b[:, cc, ch * 128:(ch + 1) * 128],
                                rhs=rhs,
                                start=(cc == 0), stop=(cc == G - 1))
                        nc.scalar.copy(qkvs[:, i, :], qps)
                    # stats + v eviction for the pair (one op each)
                    qk4 = qkvs[:, :, 0:256].rearrange("p c (g d) -> p c g d", d=64)
                    nc.vector.reduce_sum(
                        ssum_g[:, 2 * cp: 2 * cp + 2, :], qk4,
                        axis=mybir.AxisListType.X)
                    sq = ln_pool.tile([128, 2, 256], F32, tag="sq")
                    nc.scalar.square(sq, qkvs[:, :, 0:256])
                    nc.vector.reduce_sum(
                        ssq_g[:, 2 * cp: 2 * cp + 2, :],
                        sq.rearrange("p c (g d) -> p c g d", d=64),
                        axis=mybir.AxisListType.X)
                    nc.vector.tensor_copy(
                        vg[:, 2 * cp: 2 * cp + 2, :, 0:64],
                        qkvs[:, :, 256:384].rearrange("p c (h d) -> p c h d", d=64))

                # batched LN small-ops for all 8 chunks of this pair
                mu_g = st_pool.tile([128, TCH, 4], F32, tag="mu")
                nc.vector.tensor_scalar_mul(out=mu_g, in0=ssum_g, scalar1=1.0 / 64)
                rs_g = st_pool.tile([128, TCH, 4], F32, tag="rs")
                nc.vector.tensor_mul(rs_g, mu_g, mu_g)
                nc.vector.scalar_tensor_tensor(
                    out=rs_g, in0=ssq_g, scalar=1.0 / 64, in1=rs_g,
                    op0=mybir.AluOpType.mult, op1=mybir.AluOpType.subtract)
                # rsqrt via ln+exp: keeps ACT on the natural_log_exp table set
                # (same set as softmax exp) — a Sqrt call would force a ~2.7us
                # ACT table-set switch per use
                nc.scalar.activation(rs_g, rs_g, mybir.ActivationFunctionType.Ln,
                                     bias=eps_t)
                # q-groups fold the attention scale: exp(-0.5 ln v + ln(1/8))
                nc.scalar.activation(rs_g[:, :, 0:2], rs_g[:, :, 0:2],
                                     mybir.ActivationFunctionType.Exp,
                                     scale=-0.5, bias=ln8_t)
                nc.scalar.activation(rs_g[:, :, 2:4], rs_g[:, :, 2:4],
                                     mybir.ActivationFunctionType.Exp,
                                     scale=-0.5)

                for cp in range(NP):
                    qkvs = qkvs_l[cp]
                    qk4 = qkvs[:, :, 0:256].rearrange("p c (g d) -> p c g d", d=64)
                    qk_ln = ln_pool.tile([128, 2, 256], BF16, tag="qkln")
                    mu = mu_g[:, 2 * cp: 2 * cp + 2, :]
                    rs = rs_g[:, 2 * cp: 2 * cp + 2, :]
                    mu_b = _bc(mu, list(mu.ap[0:2]) + [[1, 4], [0, 64]])
                    rs_b = _bc(rs, list(rs.ap[0:2]) + [[1, 4], [0, 64]])
                    qkl4 = qk_ln[:].rearrange("p c (g d) -> p c g d", d=64)
                    nc.vector.tensor_sub(qkl4, qk4, mu_b)
                    nc.vector.tensor_mul(qkl4, qkl4, rs_b)

                    # ---- partial RoPE over the chunk pair ----
                    # products: q on DVE, k on Pool; combines: DVE
                    qk_rot = ln_pool.tile([128, 2, 256], BF16, tag="qkrot")
                    t_cc = ln_pool.tile([128, 2, 256], BF16, tag="tcc")
                    t_ss = ln_pool.tile([128, 2, 256], BF16, tag="tss")
                    ct4 = cs_t[cp][:]
                    st4 = sn_t[cp][:]
                    # [p, (c g), j, t0] views — c,g merge keeps APs at 3 free
                    # dims (TENSOR3D limit); cos/sin replicated per group with
                    # a pair-broadcast inner dim
                    lnv = qk_ln[:].rearrange("p c (g j t) -> p (c g) j t", j=32, t=2)
                    ccv = t_cc[:].rearrange("p c (g j t) -> p (c g) j t", j=32, t=2)
                    ssv = t_ss[:].rearrange("p c (g j t) -> p (c g) j t", j=32, t=2)
                    rot = qk_rot[:].rearrange("p c (g j t) -> p (c g) j t", j=32, t=2)
                    # q/k interleave as (c g) indices: q of both chunks are
                    # (cg) in {0,1,4,5}; k are {2,3,6,7} — NOT contiguous, so
                    # split per chunk-half instead: q half i = cg [4i, 4i+2)
                    for i in range(2):
                        qsl = slice(4 * i, 4 * i + 2)
                        ksl = slice(4 * i + 2, 4 * i + 4)
                        # cos/sin for chunk half i, 2 groups
                        base_c = ct4[:, i, 0:2, :]
                        base_s = st4[:, i, 0:2, :]
                        cq = _bc(base_c, [base_c.ap[0], [32, 2], [1, 32], [0, 2]])
                        sq_ = _bc(base_s, [base_s.ap[0], [32, 2], [1, 32], [0, 2]])
                        nc.vector.tensor_mul(ccv[:, qsl], lnv[:, qsl], cq)
                        nc.vector.tensor_mul(ssv[:, qsl], lnv[:, qsl], sq_)
                        nc.gpsimd.tensor_mul(ccv[:, ksl], lnv[:, ksl], cq)
                        nc.gpsimd.tensor_mul(ssv[:, ksl], lnv[:, ksl], sq_)
                    nc.vector.tensor_sub(rot[:, :, :, 0:1], ccv[:, :, :, 0:1],
                                         ssv[:, :, :, 1:2])
                    nc.vector.tensor_add(rot[:, :, :, 1:2], ssv[:, :, :, 0:1],
                                         ccv[:, :, :, 1:2])

                    # ---- transpose q/k blocks to feature-major ----
                    for i in range(2):
                        ch = 2 * cp + i
                        tpq = t_ps.tile([128, 128], BF16, tag="tps")
                        nc.tensor.transpose(tpq, qk_rot[:, i, 0:128], ident)
                        nc.vector.tensor_copy(qt[:, ch * 128:(ch + 1) * 128], tpq)
                        tpk = t_ps.tile([128, 128], BF16, tag="tps")
                        nc.tensor.transpose(tpk, qk_rot[:, i, 128:256], ident)
                        nc.vector.tensor_copy(kt[:, ch * 128:(ch + 1) * 128], tpk)

                # ---- attention for the two heads of this pair ----
                for hl in range(2):
                    ops = o_ps.tile([65, 1024], F32, tag="ops")
                    for tk in range(TCH):
                        scps = sc_ps.tile([128, 1024], F32, tag="scps")
                        for tqh in range(2):
                            # K=64 contraction: head hl lives in partition
                            # rows hl*64..hl*64+63 of kt and qt
                            nc.tensor.matmul(
                                scps[:, tqh * 512:(tqh + 1) * 512],
                                lhsT=kt[hl * 64:(hl + 1) * 64,
                                        tk * 128:(tk + 1) * 128],
                                rhs=qt[hl * 64:(hl + 1) * 64,
                                       tqh * 512:(tqh + 1) * 512],
                                start=True, stop=True)
                        pt = p_pool.tile([128, 1024], BF16, tag="pt")
                        nc.scalar.activation(pt, scps,
                                             mybir.ActivationFunctionType.Exp)
                        for tqh in range(2):
                            nc.tensor.matmul(
                                ops[:, tqh * 512:(tqh + 1) * 512],
                                lhsT=vg[:, tk, hl, :],
                                rhs=pt[:, tqh * 512:(tqh + 1) * 512],
                                start=(tk == 0), stop=(tk == TCH - 1))
                    nc.vector.tensor_copy(at_b[hl * 64:(hl + 1) * 64, g, :],
                                          ops[0:64, :])
                    # denominator row -> partition-0 tile -> DMA into dn
                    # (compute engines need aligned partition bases; DMA can
                    # write any partition)
                    drow = st_pool.tile([1, S], F32, tag="drow")
                    nc.vector.tensor_copy(drow, ops[64:65, :])
                    nc.sync.dma_start(out=dn2[hl: hl + 1, :], in_=drow)

                # ---- normalize this pair by its softmax denominators ----
                nc.vector.reciprocal(dn2[0:2, :], dn2[0:2, :])
                nc.vector.tensor_copy(dnb[0:2, :], dn2[0:2, :])
                for tqh in range(2):
                    bps = sc_ps.tile([128, 1024], F32, tag="scps")
                    nc.tensor.matmul(
                        bps[:, 0:512], lhsT=mask1[:],
                        rhs=dnb[:, tqh * 512:(tqh + 1) * 512],
                        start=True, stop=True)
                    sl = at_b[:, g, tqh * 512:(tqh + 1) * 512]
                    nc.vector.tensor_mul(sl, sl, bps[:, 0:512])

            # ---- output projection ----
            for ch in range(TCH):
                for fp in range(2):
                    pps = qkv_ps.tile([128, 384], F32, tag="qkv")
                    for cc in range(G):
                        nc.tensor.matmul(
                            pps,
                            lhsT=at_b[:, cc, ch * 128:(ch + 1) * 128],
                            rhs=wp_all[:, cc, fp * 384:(fp + 1) * 384],
                            start=(cc == 0), stop=False)
                    nc.tensor.matmul(
                        pps, lhsT=mask0[:],
                        rhs=bias_row[:, fp * 384:(fp + 1) * 384],
                        start=False, stop=True)
                    ob = ob_pool.tile([128, 384], F32, tag="ob")
                    nc.scalar.copy(ob, pps)
                    nc.sync.dma_start(
                        out=out_d[b * S + ch * 128: b * S + (ch + 1) * 128,
                                  fp * 384:(fp + 1) * 384],
                        in_=ob)


_NC_CACHE = None


def kernel(**inputs):
    global LAST_RESULT, _NC_CACHE
    x = np.ascontiguousarray(np.asarray(inputs["x"], dtype=np.float32))
    cos = np.ascontiguousarray(np.asarray(inputs["cos"], dtype=np.float32))
    sin = np.ascontiguousarray(np.asarray(inputs["sin"], dtype=np.float32))
    w_qkv = np.ascontiguousarray(np.asarray(inputs["w_qkv"], dtype=np.float32))
    w_proj = np.ascontiguousarray(np.asarray(inputs["w_proj"], dtype=np.float32))
    b_proj = np.ascontiguousarray(np.asarray(inputs["b_proj"], dtype=np.float32))

    if _NC_CACHE is None:
        _NC_CACHE = build_nc()
    nc = _NC_CACHE

    n_cores = 8
    in_maps = []
    for c in range(n_cores):
        in_maps.append({
            "x": x[B_LOC * c: B_LOC * (c + 1)].reshape(B_LOC * S, C),
            "cos": cos, "sin": sin,
            "w_qkv": w_qkv, "w_proj": w_proj, "b_proj": b_proj,
        })

    res = run_bass_kernel_spmd(
        nc, in_maps, core_ids=list(range(n_cores)),
        trace=bool(os.environ.get("BASS_TRACE")),
    )
    LAST_RESULT = res
    out = np.concatenate(
        [res.results[c]["out"].reshape(B_LOC, S, C) for c in range(n_cores)], axis=0)
    return out.astype(np.float32)

